# revision 11
# baseline (speedup 1.0000x reference)
"""Trainium2 Bass kernel for nn_GeneralizedAttention (Performer-style linear
attention with GELU random features).

Math (per (b,h)):
    qp  = gelu(q @ proj^T)            [n, m]
    kp  = gelu(k @ proj^T)            [n, m]
    ksum= kp.sum(n)                   [m]
    ctx = kp^T @ v                    [m, e]
    den = qp @ ksum                   [n]
    out = (qp @ ctx) / den[:, None]   [n, e]

Sharding: B*H = 64 (b,h) pairs split across 8 cores, 8 pairs each; proj_mat
replicated; no cross-core comms.

On-chip layouts per (b,h):
    q^T, k^T as [128, 16, 128] where partition = (t*64 + d), free = (j, p),
    n = j*256 + t*128 + p.  Both 64-row halves are used, so projection matmuls
    issue in (t=0, t=1) pairs on disjoint PE row groups and overlap.
    qp^T is kept [m, n]-major (feeds the final contraction over m),
    kp is kept [n, m]-major (feeds the context contraction over n).
    The ones column appended to v folds ksum/den into ctx/out as row 64.
"""

import numpy as np

B, H, N, D, M = 4, 16, 4096, 64, 256
NCORES = 8
BH = B * H
BHPC = BH // NCORES  # 8 (b,h) pairs per core
P = 128
NJ = N // 256        # 16 pair-blocks of 256 n
NCH = N // P         # 32 chunks of 128 n
EAUG = D + 1         # 65: e plus the folded ksum/den row


def _emit_body(ctx, tc, out_d, q_d, k_d, v_d, proj_d, bhpc, repeat=1):
    import concourse.bass as bass
    import concourse.mybir as mybir
    from concourse.masks import make_identity

    nc = tc.nc
    f32 = mybir.dt.float32
    bf16 = mybir.dt.bfloat16
    MULT = mybir.AluOpType.mult
    GELU = mybir.ActivationFunctionType.Gelu

    const = ctx.enter_context(tc.tile_pool(name="const", bufs=1))
    inp = ctx.enter_context(tc.tile_pool(name="inp", bufs=4))
    vpool = ctx.enter_context(tc.tile_pool(name="vpool", bufs=2))
    tsb = ctx.enter_context(tc.tile_pool(name="tsb", bufs=4))
    feat = ctx.enter_context(tc.tile_pool(name="feat", bufs=2))
    small = ctx.enter_context(tc.tile_pool(name="small", bufs=2))
    outp = ctx.enter_context(tc.tile_pool(name="outp", bufs=2))
    ps_gen = ctx.enter_context(tc.tile_pool(name="ps_gen", bufs=2, space="PSUM"))
    ps_small = ctx.enter_context(tc.tile_pool(name="ps_small", bufs=2, space="PSUM"))
    ps_acc = ctx.enter_context(tc.tile_pool(name="ps_acc", bufs=2, space="PSUM"))

    ident_bf = const.tile([P, P], bf16, name="ident_bf")
    make_identity(nc, ident_bf)
    ident_f32 = const.tile([P, P], f32, name="ident_f32")
    make_identity(nc, ident_f32)

    # proj^T [d, m] duplicated on both partition halves (rows 0-63 and 64-127)
    proj_nat = const.tile([P, 2, D], f32, name="proj_nat")
    nc.sync.dma_start(proj_nat[:], proj_d.rearrange("(t p) d -> p t d", p=P))
    projT = const.tile([P, M], bf16, name="projT")
    for t in range(2):
        pspt = ps_small.tile([D, P], f32, tag="small", name=f"ps_projT{t}")
        nc.tensor.transpose(pspt[:], proj_nat[:, t, :], ident_f32)
        nc.vector.tensor_copy(projT[0:D, P * t : P * (t + 1)], pspt[:])
        nc.vector.tensor_copy(projT[D:P, P * t : P * (t + 1)], pspt[:])

    if repeat > 1:
        loop_cm = tc.For_i(0, repeat, 1)
        loop_cm.__enter__()

    for bh in range(bhpc):
        # ---- loads (SWDGE cast f32 -> bf16) ----
        q_pairs = inp.tile([P, NJ, P], bf16, tag="qk", name=f"q_pairs{bh}")
        nc.gpsimd.dma_start(
            q_pairs.rearrange("p j (t d) -> p j t d", t=2),
            q_d[bh].rearrange("(j t p) d -> p j t d", t=2, p=P),
        )
        k_pairs = inp.tile([P, NJ, P], bf16, tag="qk", name=f"k_pairs{bh}")
        nc.gpsimd.dma_start(
            k_pairs.rearrange("p j (t d) -> p j t d", t=2),
            k_d[bh].rearrange("(j t p) d -> p j t d", t=2, p=P),
        )
        v_aug = vpool.tile([P, NCH, EAUG], bf16, tag="va", name=f"v_aug{bh}")
        nc.gpsimd.memset(v_aug[:, :, D:EAUG], 1.0)
        nc.gpsimd.dma_start(
            v_aug[:, :, 0:D], v_d[bh].rearrange("(c p) d -> p c d", p=P)
        )

        # ---- transpose q, k into [ (t,d), (j,p) ] ----
        qT = tsb.tile([P, NJ, P], bf16, tag="t", name=f"qT{bh}")
        kT = tsb.tile([P, NJ, P], bf16, tag="t", name=f"kT{bh}")
        for src, dst in ((q_pairs, qT), (k_pairs, kT)):
            for g in range(NJ // 8):
                pst = ps_small.tile([P, 8, P], bf16, tag="small", name=f"ps_t{bh}g{g}")
                for i in range(8):
                    nc.tensor.transpose(pst[:, i, :], src[:, 8 * g + i, :], ident_bf)
                nc.vector.tensor_copy(dst[:, 8 * g : 8 * g + 8, :], pst[:])

        # ---- qp^T = gelu(proj @ q^T)  [m, n] ----
        qpT = feat.tile([P, 2, 2, NJ, P], bf16, tag="qpT", name=f"qpT{bh}")
        for mc in range(2):
            for g in range(2):  # j in [8g, 8g+8)
                psq = [
                    ps_gen.tile([P, 1024], f32, tag="gen", name=f"ps_qp{bh}_{mc}{g}{t}")
                    for t in range(2)
                ]
                for i in range(2):
                    b4 = 2 * g + i
                    for t in range(2):
                        nc.tensor.matmul(
                            psq[t][:, 512 * i : 512 * (i + 1)],
                            lhsT=projT[64 * t : 64 * t + 64, P * mc : P * (mc + 1)],
                            rhs=qT[64 * t : 64 * t + 64, 4 * b4 : 4 * b4 + 4, :],
                        )
                for t in range(2):
                    nc.scalar.activation(
                        qpT[:, mc, t, 8 * g : 8 * g + 8, :], psq[t][:], GELU
                    )

        # ---- kp = gelu(k @ proj^T)  [n, m], interleaved with ctx accumulation ----
        kp = feat.tile([P, NCH, M], bf16, tag="kp", name=f"kp{bh}")
        # view with c = 2j + t split so gelu output order (t, jl, m) maps to chunks
        kp_v = kp.rearrange("p (j t) m -> p t j m", t=2)
        ps_ctx = ps_acc.tile([EAUG, M], f32, tag="acc", name=f"ps_ctx{bh}")
        for g in range(8):  # chunks c in [4g, 4g+4)
            psk = ps_gen.tile([P, 1024], f32, tag="gen", name=f"ps_kp{bh}_{g}")
            for jl in range(2):
                j = 2 * g + jl
                for t in range(2):
                    nc.tensor.matmul(
                        psk[:, 256 * (2 * t + jl) : 256 * (2 * t + jl + 1)],
                        lhsT=kT[64 * t : 64 * t + 64, j, :],
                        rhs=projT[64 * t : 64 * t + 64, :],
                    )
            nc.scalar.activation(kp_v[:, :, 2 * g : 2 * g + 2, :], psk[:], GELU)
            for cl in range(4):
                c = 4 * g + cl
                nc.tensor.matmul(
                    ps_ctx[:],
                    lhsT=v_aug[:, c, :],
                    rhs=kp[:, c, :],
                    start=(c == 0),
                    stop=(c == NCH - 1),
                )

        # ---- ctx^T -> ctx_aug [m, e+1] ----
        ctx_sb = small.tile([EAUG, M], bf16, tag="ctxsb", name=f"ctx_sb{bh}")
        nc.vector.tensor_copy(ctx_sb[:], ps_ctx[:])
        ctxT = small.tile([P, 2, EAUG], bf16, tag="ctxT", name=f"ctxT{bh}")
        for mc in range(2):
            psct = ps_small.tile([P, EAUG], bf16, tag="small", name=f"ps_ctxT{bh}{mc}")
            nc.tensor.transpose(
                psct[:],
                ctx_sb[:, P * mc : P * (mc + 1)],
                ident_bf[0:EAUG, 0:EAUG],
            )
            nc.vector.tensor_copy(ctxT[:, mc, :], psct[:])

        # ---- out_un^T = ctx_aug^T @ qp^T (row 64 = den), normalize, transpose ----
        out_stage = outp.tile([P, NJ, 2, D], f32, tag="ost", name=f"out_stage{bh}")
        for t in range(2):
            for jb in range(4):
                psf = ps_acc.tile([EAUG, 512], f32, tag="acc", name=f"ps_fin{bh}{t}{jb}")
                for mc in range(2):
                    nc.tensor.matmul(
                        psf[:],
                        lhsT=ctxT[:, mc, :],
                        rhs=qpT[:, mc, t, 4 * jb : 4 * jb + 4, :],
                        start=(mc == 0),
                        stop=(mc == 1),
                    )
                fin_sb = small.tile([EAUG, 512], f32, tag="fin", name=f"fin_sb{bh}{t}{jb}")
                nc.vector.tensor_copy(fin_sb[:], psf[:])
                psn = ps_small.tile([P, 4, 68], f32, tag="small", name=f"ps_n{bh}{t}{jb}")
                for i in range(4):
                    nc.tensor.transpose(
                        psn[:, i, 0:EAUG],
                        fin_sb[:, P * i : P * (i + 1)],
                        ident_f32[0:EAUG, 0:EAUG],
                    )
                rec = small.tile([P, 4], f32, tag="rec", name=f"rec{bh}{t}{jb}")
                nc.vector.reciprocal(rec[:], psn[:, :, D])
                nc.vector.tensor_tensor(
                    out_stage[:, 4 * jb : 4 * jb + 4, t, :],
                    psn[:, :, 0:D],
                    rec[:, :, None].to_broadcast((P, 4, D)),
                    MULT,
                )
        nc.sync.dma_start(
            out_d[bh].rearrange("(j t p) d -> p j t d", t=2, p=P), out_stage[:]
        )

    if repeat > 1:
        loop_cm.__exit__(None, None, None)


def build(bhpc=BHPC, repeat=1):
    from contextlib import ExitStack

    import concourse.mybir as mybir
    import concourse.tile as tile
    from concourse import bacc

    nc = bacc.Bacc("TRN2", target_bir_lowering=False, debug=False)
    f32 = mybir.dt.float32
    q_d = nc.dram_tensor("q", [bhpc, N, D], f32, kind="ExternalInput").ap()
    k_d = nc.dram_tensor("k", [bhpc, N, D], f32, kind="ExternalInput").ap()
    v_d = nc.dram_tensor("v", [bhpc, N, D], f32, kind="ExternalInput").ap()
    proj_d = nc.dram_tensor("proj_mat", [M, D], f32, kind="ExternalInput").ap()
    out_d = nc.dram_tensor("out", [bhpc, N, D], f32, kind="ExternalOutput").ap()

    with tile.TileContext(nc) as tc:
        with ExitStack() as body_ctx:
            _emit_body(body_ctx, tc, out_d, q_d, k_d, v_d, proj_d, bhpc, repeat)
    nc.compile()
    return nc


_built = None


def _get_built():
    global _built
    if _built is None:
        _built = build()
    return _built


def _shard_inputs(q, k, v, proj_mat):
    qf = np.ascontiguousarray(q.reshape(BH, N, D), dtype=np.float32)
    kf = np.ascontiguousarray(k.reshape(BH, N, D), dtype=np.float32)
    vf = np.ascontiguousarray(v.reshape(BH, N, D), dtype=np.float32)
    pf = np.ascontiguousarray(proj_mat, dtype=np.float32)
    in_maps = []
    for c in range(NCORES):
        s = slice(c * BHPC, (c + 1) * BHPC)
        in_maps.append({"q": qf[s], "k": kf[s], "v": vf[s], "proj_mat": pf})
    return in_maps


def run_on_hw(q, k, v, proj_mat, trace=False, **kwargs):
    from concourse.bass_utils import run_bass_kernel_spmd

    nc = _get_built()
    in_maps = _shard_inputs(q, k, v, proj_mat)
    res = run_bass_kernel_spmd(
        nc, in_maps, core_ids=list(range(NCORES)), trace=trace, **kwargs
    )
    out = np.concatenate([r["out"] for r in res.results], axis=0)
    return out.reshape(B, H, N, D).astype(np.float32), res


def kernel(q, k, v, proj_mat):
    out, _ = run_on_hw(q, k, v, proj_mat, trace=False)
    return out


# revision 12
# speedup vs baseline: 1.7011x; 1.7011x over previous
"""Trainium2 Bass kernel for nn_GeneralizedAttention (Performer-style linear
attention with GELU random features).

Math (per (b,h)):
    qp  = gelu(q @ proj^T)            [n, m]
    kp  = gelu(k @ proj^T)            [n, m]
    ksum= kp.sum(n)                   [m]
    ctx = kp^T @ v                    [m, e]
    den = qp @ ksum                   [n]
    out = (qp @ ctx) / den[:, None]   [n, e]

Sharding: B*H = 64 (b,h) pairs split across 8 cores, 8 pairs each; proj_mat
replicated; no cross-core comms.

On-chip layouts per (b,h):
    q^T, k^T as [128, 16, 128] where partition = (t*64 + d), free = (j, p),
    n = j*256 + t*128 + p.  Both 64-row halves are used, so projection matmuls
    issue in (t=0, t=1) pairs on disjoint PE row groups and overlap.
    qp^T is kept [m, n]-major (feeds the final contraction over m),
    kp is kept [n, m]-major (feeds the context contraction over n).
    The ones column appended to v folds ksum/den into ctx/out as row 64.
"""

import numpy as np

B, H, N, D, M = 4, 16, 4096, 64, 256
NCORES = 8
BH = B * H
BHPC = BH // NCORES  # 8 (b,h) pairs per core
P = 128
NJ = N // 256        # 16 pair-blocks of 256 n
NCH = N // P         # 32 chunks of 128 n
EAUG = D + 1         # 65: e plus the folded ksum/den row


def _emit_body(ctx, tc, out_d, q_d, k_d, v_d, proj_d, bhpc, repeat=1):
    import concourse.bass as bass
    import concourse.mybir as mybir
    from concourse.masks import make_identity

    nc = tc.nc
    f32 = mybir.dt.float32
    bf16 = mybir.dt.bfloat16
    MULT = mybir.AluOpType.mult
    GELU = mybir.ActivationFunctionType.Gelu

    const = ctx.enter_context(tc.tile_pool(name="const", bufs=1))
    inp = ctx.enter_context(tc.tile_pool(name="inp", bufs=6))
    vpool = ctx.enter_context(tc.tile_pool(name="vpool", bufs=2))
    tsb = ctx.enter_context(tc.tile_pool(name="tsb", bufs=6))
    feat = ctx.enter_context(tc.tile_pool(name="feat", bufs=2))
    small = ctx.enter_context(tc.tile_pool(name="small", bufs=2))
    outp = ctx.enter_context(tc.tile_pool(name="outp", bufs=2))
    ps_gen = ctx.enter_context(tc.tile_pool(name="ps_gen", bufs=2, space="PSUM"))
    ps_small = ctx.enter_context(tc.tile_pool(name="ps_small", bufs=2, space="PSUM"))
    ps_acc = ctx.enter_context(tc.tile_pool(name="ps_acc", bufs=2, space="PSUM"))

    ident_bf = const.tile([P, P], bf16, name="ident_bf")
    make_identity(nc, ident_bf)
    ident_f32 = const.tile([P, P], f32, name="ident_f32")
    make_identity(nc, ident_f32)

    # proj^T [d, m] duplicated on both partition halves (rows 0-63 and 64-127)
    proj_nat = const.tile([P, 2, D], f32, name="proj_nat")
    nc.sync.dma_start(proj_nat[:], proj_d.rearrange("(t p) d -> p t d", p=P))
    projT = const.tile([P, M], bf16, name="projT")
    for t in range(2):
        pspt = ps_small.tile([D, P], f32, tag="small", name=f"ps_projT{t}")
        nc.tensor.transpose(pspt[:], proj_nat[:, t, :], ident_f32)
        nc.vector.tensor_copy(projT[0:D, P * t : P * (t + 1)], pspt[:])
        nc.vector.tensor_copy(projT[D:P, P * t : P * (t + 1)], pspt[:])

    if repeat > 1:
        loop_cm = tc.For_i(0, repeat, 1)
        loop_cm.__enter__()

    for bh in range(bhpc):
        # ---- loads (SWDGE cast f32 -> bf16) ----
        q_pairs = inp.tile([P, NJ, P], bf16, tag="qk", name=f"q_pairs{bh}")
        nc.gpsimd.dma_start(
            q_pairs.rearrange("p j (t d) -> p j t d", t=2),
            q_d[bh].rearrange("(j t p) d -> p j t d", t=2, p=P),
        )
        k_pairs = inp.tile([P, NJ, P], bf16, tag="qk", name=f"k_pairs{bh}")
        nc.gpsimd.dma_start(
            k_pairs.rearrange("p j (t d) -> p j t d", t=2),
            k_d[bh].rearrange("(j t p) d -> p j t d", t=2, p=P),
        )
        v_aug = vpool.tile([P, NCH, EAUG], bf16, tag="va", name=f"v_aug{bh}")
        nc.gpsimd.memset(v_aug[:, :, D:EAUG], 1.0)
        nc.gpsimd.dma_start(
            v_aug[:, :, 0:D], v_d[bh].rearrange("(c p) d -> p c d", p=P)
        )

        # ---- transpose q, k into [ (t,d), (j,p) ] ----
        qT = tsb.tile([P, NJ, P], bf16, tag="t", name=f"qT{bh}")
        kT = tsb.tile([P, NJ, P], bf16, tag="t", name=f"kT{bh}")
        for src, dst in ((q_pairs, qT), (k_pairs, kT)):
            for g in range(NJ // 8):
                pst = ps_small.tile([P, 8, P], bf16, tag="small", name=f"ps_t{bh}g{g}")
                for i in range(8):
                    nc.tensor.transpose(pst[:, i, :], src[:, 8 * g + i, :], ident_bf)
                nc.vector.tensor_copy(dst[:, 8 * g : 8 * g + 8, :], pst[:])

        # ---- qp^T = gelu(proj @ q^T)  [m, n] ----
        qpT = feat.tile([P, 2, 2, NJ, P], bf16, tag="qpT", name=f"qpT{bh}")
        for mc in range(2):
            for g in range(2):  # j in [8g, 8g+8)
                psq = [
                    ps_gen.tile([P, 1024], f32, tag="gen", name=f"ps_qp{bh}_{mc}{g}{t}")
                    for t in range(2)
                ]
                for i in range(2):
                    b4 = 2 * g + i
                    for t in range(2):
                        nc.tensor.matmul(
                            psq[t][:, 512 * i : 512 * (i + 1)],
                            lhsT=projT[64 * t : 64 * t + 64, P * mc : P * (mc + 1)],
                            rhs=qT[64 * t : 64 * t + 64, 4 * b4 : 4 * b4 + 4, :],
                        )
                for t in range(2):
                    nc.scalar.activation(
                        qpT[:, mc, t, 8 * g : 8 * g + 8, :], psq[t][:], GELU
                    )

        # ---- kp = gelu(k @ proj^T)  [n, m], interleaved with ctx accumulation ----
        kp = feat.tile([P, NCH, M], bf16, tag="kp", name=f"kp{bh}")
        # view with c = 2j + t split so gelu output order (t, jl, m) maps to chunks
        kp_v = kp.rearrange("p (j t) m -> p t j m", t=2)
        ps_ctx = ps_acc.tile([EAUG, M], f32, tag="acc", name=f"ps_ctx{bh}")
        for g in range(8):  # chunks c in [4g, 4g+4)
            psk = ps_gen.tile([P, 1024], f32, tag="gen", name=f"ps_kp{bh}_{g}")
            for jl in range(2):
                j = 2 * g + jl
                for t in range(2):
                    nc.tensor.matmul(
                        psk[:, 256 * (2 * t + jl) : 256 * (2 * t + jl + 1)],
                        lhsT=kT[64 * t : 64 * t + 64, j, :],
                        rhs=projT[64 * t : 64 * t + 64, :],
                    )
            nc.scalar.activation(kp_v[:, :, 2 * g : 2 * g + 2, :], psk[:], GELU)
            for cl in range(4):
                c = 4 * g + cl
                nc.tensor.matmul(
                    ps_ctx[:],
                    lhsT=v_aug[:, c, :],
                    rhs=kp[:, c, :],
                    start=(c == 0),
                    stop=(c == NCH - 1),
                )

        # ---- ctx^T -> ctx_aug [m, e+1] ----
        ctx_sb = small.tile([EAUG, M], bf16, tag="ctxsb", name=f"ctx_sb{bh}")
        nc.vector.tensor_copy(ctx_sb[:], ps_ctx[:])
        ctxT = small.tile([P, 2, EAUG], bf16, tag="ctxT", name=f"ctxT{bh}")
        for mc in range(2):
            psct = ps_small.tile([P, EAUG], bf16, tag="small", name=f"ps_ctxT{bh}{mc}")
            nc.tensor.transpose(
                psct[:],
                ctx_sb[:, P * mc : P * (mc + 1)],
                ident_bf[0:EAUG, 0:EAUG],
            )
            nc.vector.tensor_copy(ctxT[:, mc, :], psct[:])

        # ---- out_un^T = ctx_aug^T @ qp^T (row 64 = den), normalize, transpose ----
        out_stage = outp.tile([P, NJ, 2, D], f32, tag="ost", name=f"out_stage{bh}")
        for t in range(2):
            for jb in range(4):
                psf = ps_acc.tile([EAUG, 512], f32, tag="acc", name=f"ps_fin{bh}{t}{jb}")
                for mc in range(2):
                    nc.tensor.matmul(
                        psf[:],
                        lhsT=ctxT[:, mc, :],
                        rhs=qpT[:, mc, t, 4 * jb : 4 * jb + 4, :],
                        start=(mc == 0),
                        stop=(mc == 1),
                    )
                fin_sb = small.tile([EAUG, 512], f32, tag="fin", name=f"fin_sb{bh}{t}{jb}")
                nc.vector.tensor_copy(fin_sb[:], psf[:])
                psn = ps_small.tile([P, 4, 68], f32, tag="small", name=f"ps_n{bh}{t}{jb}")
                for i in range(4):
                    nc.tensor.transpose(
                        psn[:, i, 0:EAUG],
                        fin_sb[:, P * i : P * (i + 1)],
                        ident_f32[0:EAUG, 0:EAUG],
                    )
                rec = small.tile([P, 4], f32, tag="rec", name=f"rec{bh}{t}{jb}")
                nc.vector.reciprocal(rec[:], psn[:, :, D])
                nc.vector.tensor_tensor(
                    out_stage[:, 4 * jb : 4 * jb + 4, t, :],
                    psn[:, :, 0:D],
                    rec[:, :, None].to_broadcast((P, 4, D)),
                    MULT,
                )
        nc.sync.dma_start(
            out_d[bh].rearrange("(j t p) d -> p j t d", t=2, p=P), out_stage[:]
        )

    if repeat > 1:
        loop_cm.__exit__(None, None, None)


def build(bhpc=BHPC, repeat=1):
    from contextlib import ExitStack

    import concourse.mybir as mybir
    import concourse.tile as tile
    from concourse import bacc

    nc = bacc.Bacc("TRN2", target_bir_lowering=False, debug=False)
    f32 = mybir.dt.float32
    q_d = nc.dram_tensor("q", [bhpc, N, D], f32, kind="ExternalInput").ap()
    k_d = nc.dram_tensor("k", [bhpc, N, D], f32, kind="ExternalInput").ap()
    v_d = nc.dram_tensor("v", [bhpc, N, D], f32, kind="ExternalInput").ap()
    proj_d = nc.dram_tensor("proj_mat", [M, D], f32, kind="ExternalInput").ap()
    out_d = nc.dram_tensor("out", [bhpc, N, D], f32, kind="ExternalOutput").ap()

    with tile.TileContext(nc) as tc:
        with ExitStack() as body_ctx:
            _emit_body(body_ctx, tc, out_d, q_d, k_d, v_d, proj_d, bhpc, repeat)
    nc.compile()
    return nc


_built = None


def _get_built():
    global _built
    if _built is None:
        _built = build()
    return _built


def _shard_inputs(q, k, v, proj_mat):
    qf = np.ascontiguousarray(q.reshape(BH, N, D), dtype=np.float32)
    kf = np.ascontiguousarray(k.reshape(BH, N, D), dtype=np.float32)
    vf = np.ascontiguousarray(v.reshape(BH, N, D), dtype=np.float32)
    pf = np.ascontiguousarray(proj_mat, dtype=np.float32)
    in_maps = []
    for c in range(NCORES):
        s = slice(c * BHPC, (c + 1) * BHPC)
        in_maps.append({"q": qf[s], "k": kf[s], "v": vf[s], "proj_mat": pf})
    return in_maps


def run_on_hw(q, k, v, proj_mat, trace=False, **kwargs):
    from concourse.bass_utils import run_bass_kernel_spmd

    nc = _get_built()
    in_maps = _shard_inputs(q, k, v, proj_mat)
    res = run_bass_kernel_spmd(
        nc, in_maps, core_ids=list(range(NCORES)), trace=trace, **kwargs
    )
    out = np.concatenate([r["out"] for r in res.results], axis=0)
    return out.reshape(B, H, N, D).astype(np.float32), res


def kernel(q, k, v, proj_mat):
    out, _ = run_on_hw(q, k, v, proj_mat, trace=False)
    return out


# revision 13
# speedup vs baseline: 1.7156x; 1.0085x over previous
"""Trainium2 Bass kernel for nn_GeneralizedAttention (Performer-style linear
attention with GELU random features).

Math (per (b,h)):
    qp  = gelu(q @ proj^T)            [n, m]
    kp  = gelu(k @ proj^T)            [n, m]
    ksum= kp.sum(n)                   [m]
    ctx = kp^T @ v                    [m, e]
    den = qp @ ksum                   [n]
    out = (qp @ ctx) / den[:, None]   [n, e]

Sharding: B*H = 64 (b,h) pairs split across 8 cores, 8 pairs each; proj_mat
replicated; no cross-core comms.

On-chip layouts per (b,h):
    q^T, k^T as [128, 16, 128] where partition = (t*64 + d), free = (j, p),
    n = j*256 + t*128 + p.  Both 64-row halves are used, so projection matmuls
    issue in (t=0, t=1) pairs on disjoint PE row groups and overlap.
    qp^T is kept [m, n]-major (feeds the final contraction over m),
    kp is kept [n, m]-major (feeds the context contraction over n).
    The ones column appended to v folds ksum/den into ctx/out as row 64.
"""

import numpy as np

B, H, N, D, M = 4, 16, 4096, 64, 256
NCORES = 8
BH = B * H
BHPC = BH // NCORES  # 8 (b,h) pairs per core
P = 128
NJ = N // 256        # 16 pair-blocks of 256 n
NCH = N // P         # 32 chunks of 128 n
EAUG = D + 1         # 65: e plus the folded ksum/den row


def _emit_body(ctx, tc, out_d, q_d, k_d, v_d, proj_d, bhpc, repeat=1):
    import concourse.bass as bass
    import concourse.mybir as mybir
    from concourse.masks import make_identity

    nc = tc.nc
    f32 = mybir.dt.float32
    bf16 = mybir.dt.bfloat16
    MULT = mybir.AluOpType.mult
    GELU = mybir.ActivationFunctionType.Gelu

    const = ctx.enter_context(tc.tile_pool(name="const", bufs=1))
    inp = ctx.enter_context(tc.tile_pool(name="inp", bufs=6))
    vpool = ctx.enter_context(tc.tile_pool(name="vpool", bufs=3))
    tsb = ctx.enter_context(tc.tile_pool(name="tsb", bufs=6))
    feat = ctx.enter_context(tc.tile_pool(name="feat", bufs=2))
    small = ctx.enter_context(tc.tile_pool(name="small", bufs=3))
    outp = ctx.enter_context(tc.tile_pool(name="outp", bufs=3))
    ps_gen = ctx.enter_context(tc.tile_pool(name="ps_gen", bufs=2, space="PSUM"))
    ps_small = ctx.enter_context(tc.tile_pool(name="ps_small", bufs=2, space="PSUM"))
    ps_acc = ctx.enter_context(tc.tile_pool(name="ps_acc", bufs=2, space="PSUM"))

    ident_bf = const.tile([P, P], bf16, name="ident_bf")
    make_identity(nc, ident_bf)
    ident_f32 = const.tile([P, P], f32, name="ident_f32")
    make_identity(nc, ident_f32)

    # proj^T [d, m] duplicated on both partition halves (rows 0-63 and 64-127)
    proj_nat = const.tile([P, 2, D], f32, name="proj_nat")
    nc.sync.dma_start(proj_nat[:], proj_d.rearrange("(t p) d -> p t d", p=P))
    projT = const.tile([P, M], bf16, name="projT")
    for t in range(2):
        pspt = ps_small.tile([D, P], f32, tag="small", name=f"ps_projT{t}")
        nc.tensor.transpose(pspt[:], proj_nat[:, t, :], ident_f32)
        nc.vector.tensor_copy(projT[0:D, P * t : P * (t + 1)], pspt[:])
        nc.vector.tensor_copy(projT[D:P, P * t : P * (t + 1)], pspt[:])

    if repeat > 1:
        loop_cm = tc.For_i(0, repeat, 1)
        loop_cm.__enter__()

    for bh in range(bhpc):
        # ---- loads (SWDGE cast f32 -> bf16) ----
        q_pairs = inp.tile([P, NJ, P], bf16, tag="qk", name=f"q_pairs{bh}")
        nc.gpsimd.dma_start(
            q_pairs.rearrange("p j (t d) -> p j t d", t=2),
            q_d[bh].rearrange("(j t p) d -> p j t d", t=2, p=P),
        )
        k_pairs = inp.tile([P, NJ, P], bf16, tag="qk", name=f"k_pairs{bh}")
        nc.gpsimd.dma_start(
            k_pairs.rearrange("p j (t d) -> p j t d", t=2),
            k_d[bh].rearrange("(j t p) d -> p j t d", t=2, p=P),
        )
        v_aug = vpool.tile([P, NCH, EAUG], bf16, tag="va", name=f"v_aug{bh}")
        nc.gpsimd.memset(v_aug[:, :, D:EAUG], 1.0)
        nc.gpsimd.dma_start(
            v_aug[:, :, 0:D], v_d[bh].rearrange("(c p) d -> p c d", p=P)
        )

        # ---- transpose q, k into [ (t,d), (j,p) ] ----
        qT = tsb.tile([P, NJ, P], bf16, tag="t", name=f"qT{bh}")
        kT = tsb.tile([P, NJ, P], bf16, tag="t", name=f"kT{bh}")
        for src, dst in ((q_pairs, qT), (k_pairs, kT)):
            for g in range(NJ // 8):
                pst = ps_small.tile([P, 8, P], bf16, tag="small", name=f"ps_t{bh}g{g}")
                for i in range(8):
                    nc.tensor.transpose(pst[:, i, :], src[:, 8 * g + i, :], ident_bf)
                nc.vector.tensor_copy(dst[:, 8 * g : 8 * g + 8, :], pst[:])

        # ---- qp^T = gelu(proj @ q^T)  [m, n] ----
        qpT = feat.tile([P, 2, 2, NJ, P], bf16, tag="qpT", name=f"qpT{bh}")
        for mc in range(2):
            for g in range(2):  # j in [8g, 8g+8)
                psq = [
                    ps_gen.tile([P, 1024], f32, tag="gen", name=f"ps_qp{bh}_{mc}{g}{t}")
                    for t in range(2)
                ]
                for i in range(2):
                    b4 = 2 * g + i
                    for t in range(2):
                        nc.tensor.matmul(
                            psq[t][:, 512 * i : 512 * (i + 1)],
                            lhsT=projT[64 * t : 64 * t + 64, P * mc : P * (mc + 1)],
                            rhs=qT[64 * t : 64 * t + 64, 4 * b4 : 4 * b4 + 4, :],
                        )
                for t in range(2):
                    nc.scalar.activation(
                        qpT[:, mc, t, 8 * g : 8 * g + 8, :], psq[t][:], GELU
                    )

        # ---- kp = gelu(k @ proj^T)  [n, m], interleaved with ctx accumulation ----
        kp = feat.tile([P, NCH, M], bf16, tag="kp", name=f"kp{bh}")
        # view with c = 2j + t split so gelu output order (t, jl, m) maps to chunks
        kp_v = kp.rearrange("p (j t) m -> p t j m", t=2)
        ps_ctx = ps_acc.tile([EAUG, M], f32, tag="acc", name=f"ps_ctx{bh}")
        for g in range(8):  # chunks c in [4g, 4g+4)
            psk = ps_gen.tile([P, 1024], f32, tag="gen", name=f"ps_kp{bh}_{g}")
            for jl in range(2):
                j = 2 * g + jl
                for t in range(2):
                    nc.tensor.matmul(
                        psk[:, 256 * (2 * t + jl) : 256 * (2 * t + jl + 1)],
                        lhsT=kT[64 * t : 64 * t + 64, j, :],
                        rhs=projT[64 * t : 64 * t + 64, :],
                    )
            nc.scalar.activation(kp_v[:, :, 2 * g : 2 * g + 2, :], psk[:], GELU)
            for cl in range(4):
                c = 4 * g + cl
                nc.tensor.matmul(
                    ps_ctx[:],
                    lhsT=v_aug[:, c, :],
                    rhs=kp[:, c, :],
                    start=(c == 0),
                    stop=(c == NCH - 1),
                )

        # ---- ctx^T -> ctx_aug [m, e+1] ----
        ctx_sb = small.tile([EAUG, M], bf16, tag="ctxsb", name=f"ctx_sb{bh}")
        nc.vector.tensor_copy(ctx_sb[:], ps_ctx[:])
        ctxT = small.tile([P, 2, EAUG], bf16, tag="ctxT", name=f"ctxT{bh}")
        for mc in range(2):
            psct = ps_small.tile([P, EAUG], bf16, tag="small", name=f"ps_ctxT{bh}{mc}")
            nc.tensor.transpose(
                psct[:],
                ctx_sb[:, P * mc : P * (mc + 1)],
                ident_bf[0:EAUG, 0:EAUG],
            )
            nc.vector.tensor_copy(ctxT[:, mc, :], psct[:])

        # ---- out_un^T = ctx_aug^T @ qp^T (row 64 = den), normalize, transpose ----
        out_stage = outp.tile([P, NJ, 2, D], f32, tag="ost", name=f"out_stage{bh}")
        for t in range(2):
            for jb in range(4):
                psf = ps_acc.tile([EAUG, 512], f32, tag="acc", name=f"ps_fin{bh}{t}{jb}")
                for mc in range(2):
                    nc.tensor.matmul(
                        psf[:],
                        lhsT=ctxT[:, mc, :],
                        rhs=qpT[:, mc, t, 4 * jb : 4 * jb + 4, :],
                        start=(mc == 0),
                        stop=(mc == 1),
                    )
                fin_sb = small.tile([EAUG, 512], f32, tag="fin", name=f"fin_sb{bh}{t}{jb}")
                nc.vector.tensor_copy(fin_sb[:], psf[:])
                psn = ps_small.tile([P, 4, 68], f32, tag="small", name=f"ps_n{bh}{t}{jb}")
                for i in range(4):
                    nc.tensor.transpose(
                        psn[:, i, 0:EAUG],
                        fin_sb[:, P * i : P * (i + 1)],
                        ident_f32[0:EAUG, 0:EAUG],
                    )
                rec = small.tile([P, 4], f32, tag="rec", name=f"rec{bh}{t}{jb}")
                nc.vector.reciprocal(rec[:], psn[:, :, D])
                nc.vector.tensor_tensor(
                    out_stage[:, 4 * jb : 4 * jb + 4, t, :],
                    psn[:, :, 0:D],
                    rec[:, :, None].to_broadcast((P, 4, D)),
                    MULT,
                )
        nc.sync.dma_start(
            out_d[bh].rearrange("(j t p) d -> p j t d", t=2, p=P), out_stage[:]
        )

    if repeat > 1:
        loop_cm.__exit__(None, None, None)


def build(bhpc=BHPC, repeat=1):
    from contextlib import ExitStack

    import concourse.mybir as mybir
    import concourse.tile as tile
    from concourse import bacc

    nc = bacc.Bacc("TRN2", target_bir_lowering=False, debug=False)
    f32 = mybir.dt.float32
    q_d = nc.dram_tensor("q", [bhpc, N, D], f32, kind="ExternalInput").ap()
    k_d = nc.dram_tensor("k", [bhpc, N, D], f32, kind="ExternalInput").ap()
    v_d = nc.dram_tensor("v", [bhpc, N, D], f32, kind="ExternalInput").ap()
    proj_d = nc.dram_tensor("proj_mat", [M, D], f32, kind="ExternalInput").ap()
    out_d = nc.dram_tensor("out", [bhpc, N, D], f32, kind="ExternalOutput").ap()

    with tile.TileContext(nc) as tc:
        with ExitStack() as body_ctx:
            _emit_body(body_ctx, tc, out_d, q_d, k_d, v_d, proj_d, bhpc, repeat)
    nc.compile()
    return nc


_built = None


def _get_built():
    global _built
    if _built is None:
        _built = build()
    return _built


def _shard_inputs(q, k, v, proj_mat):
    qf = np.ascontiguousarray(q.reshape(BH, N, D), dtype=np.float32)
    kf = np.ascontiguousarray(k.reshape(BH, N, D), dtype=np.float32)
    vf = np.ascontiguousarray(v.reshape(BH, N, D), dtype=np.float32)
    pf = np.ascontiguousarray(proj_mat, dtype=np.float32)
    in_maps = []
    for c in range(NCORES):
        s = slice(c * BHPC, (c + 1) * BHPC)
        in_maps.append({"q": qf[s], "k": kf[s], "v": vf[s], "proj_mat": pf})
    return in_maps


def run_on_hw(q, k, v, proj_mat, trace=False, **kwargs):
    from concourse.bass_utils import run_bass_kernel_spmd

    nc = _get_built()
    in_maps = _shard_inputs(q, k, v, proj_mat)
    res = run_bass_kernel_spmd(
        nc, in_maps, core_ids=list(range(NCORES)), trace=trace, **kwargs
    )
    out = np.concatenate([r["out"] for r in res.results], axis=0)
    return out.reshape(B, H, N, D).astype(np.float32), res


def kernel(q, k, v, proj_mat):
    out, _ = run_on_hw(q, k, v, proj_mat, trace=False)
    return out


# revision 14
# speedup vs baseline: 1.7874x; 1.0418x over previous
"""Trainium2 Bass kernel for nn_GeneralizedAttention (Performer-style linear
attention with GELU random features).

Math (per (b,h)):
    qp  = gelu(q @ proj^T)            [n, m]
    kp  = gelu(k @ proj^T)            [n, m]
    ksum= kp.sum(n)                   [m]
    ctx = kp^T @ v                    [m, e]
    den = qp @ ksum                   [n]
    out = (qp @ ctx) / den[:, None]   [n, e]

Sharding: B*H = 64 (b,h) pairs split across 8 cores, 8 pairs each; proj_mat
replicated; no cross-core comms.

On-chip layouts per (b,h):
    q^T, k^T as [128, 16, 128] where partition = (t*64 + d), free = (j, p),
    n = j*256 + t*128 + p.  Both 64-row halves are used, so projection matmuls
    issue in (t=0, t=1) pairs on disjoint PE row groups and overlap.
    qp^T is kept [m, n]-major (feeds the final contraction over m),
    kp is kept [n, m]-major (feeds the context contraction over n).
    The ones column appended to v folds ksum/den into ctx/out as row 64.
"""

import numpy as np

B, H, N, D, M = 4, 16, 4096, 64, 256
NCORES = 8
BH = B * H
BHPC = BH // NCORES  # 8 (b,h) pairs per core
P = 128
NJ = N // 256        # 16 pair-blocks of 256 n
NCH = N // P         # 32 chunks of 128 n
EAUG = D + 1         # 65: e plus the folded ksum/den row


def _emit_body(ctx, tc, out_d, q_d, k_d, v_d, proj_d, bhpc, repeat=1):
    import concourse.bass as bass
    import concourse.mybir as mybir
    from concourse.masks import make_identity

    nc = tc.nc
    f32 = mybir.dt.float32
    bf16 = mybir.dt.bfloat16
    MULT = mybir.AluOpType.mult
    GELU = mybir.ActivationFunctionType.Gelu

    const = ctx.enter_context(tc.tile_pool(name="const", bufs=1))
    inp = ctx.enter_context(tc.tile_pool(name="inp", bufs=6))
    vpool = ctx.enter_context(tc.tile_pool(name="vpool", bufs=3))
    tsb = ctx.enter_context(tc.tile_pool(name="tsb", bufs=6))
    feat = ctx.enter_context(tc.tile_pool(name="feat", bufs=2))
    small = ctx.enter_context(tc.tile_pool(name="small", bufs=3))
    outp = ctx.enter_context(tc.tile_pool(name="outp", bufs=3))
    ps_gen = ctx.enter_context(tc.tile_pool(name="ps_gen", bufs=2, space="PSUM"))
    ps_small = ctx.enter_context(tc.tile_pool(name="ps_small", bufs=2, space="PSUM"))
    ps_acc = ctx.enter_context(tc.tile_pool(name="ps_acc", bufs=2, space="PSUM"))

    ident_bf = const.tile([P, P], bf16, name="ident_bf")
    make_identity(nc, ident_bf)
    ident_f32 = const.tile([P, P], f32, name="ident_f32")
    make_identity(nc, ident_f32)

    # proj^T [d, m] duplicated on both partition halves (rows 0-63 and 64-127)
    proj_nat = const.tile([P, 2, D], f32, name="proj_nat")
    nc.sync.dma_start(proj_nat[:], proj_d.rearrange("(t p) d -> p t d", p=P))
    projT = const.tile([P, M], bf16, name="projT")
    for t in range(2):
        pspt = ps_small.tile([D, P], f32, tag="small", name=f"ps_projT{t}")
        nc.tensor.transpose(pspt[:], proj_nat[:, t, :], ident_f32)
        nc.vector.tensor_copy(projT[0:D, P * t : P * (t + 1)], pspt[:])
        nc.vector.tensor_copy(projT[D:P, P * t : P * (t + 1)], pspt[:])

    if repeat > 1:
        loop_cm = tc.For_i(0, repeat, 1)
        loop_cm.__enter__()

    for bh in range(bhpc):
        # ---- loads (SWDGE cast f32 -> bf16) ----
        q_pairs = inp.tile([P, NJ, P], bf16, tag="qk", name=f"q_pairs{bh}")
        nc.gpsimd.dma_start(
            q_pairs.rearrange("p j (t d) -> p j t d", t=2),
            q_d[bh].rearrange("(j t p) d -> p j t d", t=2, p=P),
        )
        k_pairs = inp.tile([P, NJ, P], bf16, tag="qk", name=f"k_pairs{bh}")
        nc.gpsimd.dma_start(
            k_pairs.rearrange("p j (t d) -> p j t d", t=2),
            k_d[bh].rearrange("(j t p) d -> p j t d", t=2, p=P),
        )
        v_aug = vpool.tile([P, NCH, EAUG], bf16, tag="va", name=f"v_aug{bh}")
        nc.gpsimd.memset(v_aug[:, :, D:EAUG], 1.0)
        nc.gpsimd.dma_start(
            v_aug[:, :, 0:D], v_d[bh].rearrange("(c p) d -> p c d", p=P)
        )

        # ---- transpose q, k into [ (t,d), (j,p) ] ----
        qT = tsb.tile([P, NJ, P], bf16, tag="t", name=f"qT{bh}")
        kT = tsb.tile([P, NJ, P], bf16, tag="t", name=f"kT{bh}")
        for src, dst in ((q_pairs, qT), (k_pairs, kT)):
            for g in range(NJ // 8):
                pst = ps_small.tile([P, 8, P], bf16, tag="small", name=f"ps_t{bh}g{g}")
                for i in range(8):
                    nc.tensor.transpose(pst[:, i, :], src[:, 8 * g + i, :], ident_bf)
                nc.vector.tensor_copy(dst[:, 8 * g : 8 * g + 8, :], pst[:])

        # ---- qp^T = gelu(proj @ q^T)  [m, n] ----
        qpT = feat.tile([P, 2, 2, NJ, P], bf16, tag="qpT", name=f"qpT{bh}")
        for mc in range(2):
            for b4 in range(4):  # j in [4b4, 4b4+4)
                psq = ps_gen.tile([P, 1024], f32, tag="gen", name=f"ps_qp{bh}_{mc}{b4}")
                for t in range(2):
                    # t=0 -> bank A (cols 0:512), t=1 -> bank B: the pair can
                    # stream concurrently on disjoint row groups/banks
                    nc.tensor.matmul(
                        psq[:, 512 * t : 512 * (t + 1)],
                        lhsT=projT[64 * t : 64 * t + 64, P * mc : P * (mc + 1)],
                        rhs=qT[64 * t : 64 * t + 64, 4 * b4 : 4 * b4 + 4, :],
                    )
                nc.scalar.activation(
                    qpT[:, mc, :, 4 * b4 : 4 * b4 + 4, :], psq[:], GELU
                )

        # ---- kp = gelu(k @ proj^T)  [n, m], interleaved with ctx accumulation ----
        kp = feat.tile([P, NCH, M], bf16, tag="kp", name=f"kp{bh}")
        # view with c = 2j + t split so gelu output order (t, jl, m) maps to chunks
        kp_v = kp.rearrange("p (j t) m -> p t j m", t=2)
        ps_ctx = ps_acc.tile([EAUG, M], f32, tag="acc", name=f"ps_ctx{bh}")
        for g in range(8):  # chunks c in [4g, 4g+4)
            psk = ps_gen.tile([P, 1024], f32, tag="gen", name=f"ps_kp{bh}_{g}")
            for jl in range(2):
                j = 2 * g + jl
                for t in range(2):
                    nc.tensor.matmul(
                        psk[:, 256 * (2 * t + jl) : 256 * (2 * t + jl + 1)],
                        lhsT=kT[64 * t : 64 * t + 64, j, :],
                        rhs=projT[64 * t : 64 * t + 64, :],
                    )
            nc.scalar.activation(kp_v[:, :, 2 * g : 2 * g + 2, :], psk[:], GELU)
            for cl in range(4):
                c = 4 * g + cl
                nc.tensor.matmul(
                    ps_ctx[:],
                    lhsT=v_aug[:, c, :],
                    rhs=kp[:, c, :],
                    start=(c == 0),
                    stop=(c == NCH - 1),
                )

        # ---- ctx^T -> ctx_aug [m, e+1] ----
        ctx_sb = small.tile([EAUG, M], bf16, tag="ctxsb", name=f"ctx_sb{bh}")
        nc.vector.tensor_copy(ctx_sb[:], ps_ctx[:])
        ctxT = small.tile([P, 2, EAUG], bf16, tag="ctxT", name=f"ctxT{bh}")
        for mc in range(2):
            psct = ps_small.tile([P, EAUG], bf16, tag="small", name=f"ps_ctxT{bh}{mc}")
            nc.tensor.transpose(
                psct[:],
                ctx_sb[:, P * mc : P * (mc + 1)],
                ident_bf[0:EAUG, 0:EAUG],
            )
            nc.vector.tensor_copy(ctxT[:, mc, :], psct[:])

        # ---- out_un^T = ctx_aug^T @ qp^T (row 64 = den), normalize, transpose ----
        out_stage = outp.tile([P, NJ, 2, D], f32, tag="ost", name=f"out_stage{bh}")
        for t in range(2):
            for jb in range(4):
                psf = ps_acc.tile([EAUG, 512], f32, tag="acc", name=f"ps_fin{bh}{t}{jb}")
                for mc in range(2):
                    nc.tensor.matmul(
                        psf[:],
                        lhsT=ctxT[:, mc, :],
                        rhs=qpT[:, mc, t, 4 * jb : 4 * jb + 4, :],
                        start=(mc == 0),
                        stop=(mc == 1),
                    )
                fin_sb = small.tile([EAUG, 512], f32, tag="fin", name=f"fin_sb{bh}{t}{jb}")
                nc.vector.tensor_copy(fin_sb[:], psf[:])
                psn = ps_small.tile([P, 4, 68], f32, tag="small", name=f"ps_n{bh}{t}{jb}")
                for i in range(4):
                    nc.tensor.transpose(
                        psn[:, i, 0:EAUG],
                        fin_sb[:, P * i : P * (i + 1)],
                        ident_f32[0:EAUG, 0:EAUG],
                    )
                rec = small.tile([P, 4], f32, tag="rec", name=f"rec{bh}{t}{jb}")
                nc.vector.reciprocal(rec[:], psn[:, :, D])
                nc.vector.tensor_tensor(
                    out_stage[:, 4 * jb : 4 * jb + 4, t, :],
                    psn[:, :, 0:D],
                    rec[:, :, None].to_broadcast((P, 4, D)),
                    MULT,
                )
        nc.sync.dma_start(
            out_d[bh].rearrange("(j t p) d -> p j t d", t=2, p=P), out_stage[:]
        )

    if repeat > 1:
        loop_cm.__exit__(None, None, None)


def build(bhpc=BHPC, repeat=1):
    from contextlib import ExitStack

    import concourse.mybir as mybir
    import concourse.tile as tile
    from concourse import bacc

    nc = bacc.Bacc("TRN2", target_bir_lowering=False, debug=False)
    f32 = mybir.dt.float32
    q_d = nc.dram_tensor("q", [bhpc, N, D], f32, kind="ExternalInput").ap()
    k_d = nc.dram_tensor("k", [bhpc, N, D], f32, kind="ExternalInput").ap()
    v_d = nc.dram_tensor("v", [bhpc, N, D], f32, kind="ExternalInput").ap()
    proj_d = nc.dram_tensor("proj_mat", [M, D], f32, kind="ExternalInput").ap()
    out_d = nc.dram_tensor("out", [bhpc, N, D], f32, kind="ExternalOutput").ap()

    with tile.TileContext(nc) as tc:
        with ExitStack() as body_ctx:
            _emit_body(body_ctx, tc, out_d, q_d, k_d, v_d, proj_d, bhpc, repeat)
    nc.compile()
    return nc


_built = None


def _get_built():
    global _built
    if _built is None:
        _built = build()
    return _built


def _shard_inputs(q, k, v, proj_mat):
    qf = np.ascontiguousarray(q.reshape(BH, N, D), dtype=np.float32)
    kf = np.ascontiguousarray(k.reshape(BH, N, D), dtype=np.float32)
    vf = np.ascontiguousarray(v.reshape(BH, N, D), dtype=np.float32)
    pf = np.ascontiguousarray(proj_mat, dtype=np.float32)
    in_maps = []
    for c in range(NCORES):
        s = slice(c * BHPC, (c + 1) * BHPC)
        in_maps.append({"q": qf[s], "k": kf[s], "v": vf[s], "proj_mat": pf})
    return in_maps


def run_on_hw(q, k, v, proj_mat, trace=False, **kwargs):
    from concourse.bass_utils import run_bass_kernel_spmd

    nc = _get_built()
    in_maps = _shard_inputs(q, k, v, proj_mat)
    res = run_bass_kernel_spmd(
        nc, in_maps, core_ids=list(range(NCORES)), trace=trace, **kwargs
    )
    out = np.concatenate([r["out"] for r in res.results], axis=0)
    return out.reshape(B, H, N, D).astype(np.float32), res


def kernel(q, k, v, proj_mat):
    out, _ = run_on_hw(q, k, v, proj_mat, trace=False)
    return out
